# revision 38
# baseline (speedup 1.0000x reference)
"""Multi-head attention (Keras-style, relu-activated dense projections)
for Trainium2, SPMD across 8 NeuronCores.

Problem (full shapes):
    B, S, D, H = 4, 1024, 1024, 16 ; DH = 64
    qp = relu(q @ Wq + bq); kp = relu(k @ Wk + bk); vp = relu(v @ Wv + bv)
    per head h: scores = qh @ kh^T / 8 ; attn = softmax(scores)
    out = relu(concat_h(attn @ vh) @ Wo + bo)

Sharding: core c = (batch b = c//2, head-group g = c%2). Each core computes
the 8 heads of group g for batch b end-to-end and produces the partial
output projection  attn_out_g @ Wo[g*512:(g+1)*512, :]  (no bias / relu).
Host sums the two partials per batch, adds bo, applies relu.

v3 design notes:
  - all matmul operands bf16 (host-cast): halves input DMA, FWL weight
    loads, no fp32_mode=HIGH matmuls.
  - inputs land via ~30 large chunked DMAs (512KB) instead of ~95 small
    ones: each dma_start costs ~600ns of serialized HWDGE issue, which
    paced the whole projection era in v2.
  - attention software-pipelined across 8 (pc,hp) blocks: scores+exp of
    block i emitted before attn@v/Z/normalize of block i-1, V projection
    and out-projection emitted as PE filler; Tile's readiness scheduler
    keeps ACT saturated with exp and PE dense (HAM stays warm).
  - engine balance: exp + QK bias-relu + Z staging copy on ACT; tree-sum,
    V relu, reciprocal, normalize, out copies on DVE.
  - partial outputs returned bf16; host combines in fp32.
"""

import numpy as np
import ml_dtypes
from contextlib import ExitStack

import concourse.bass as bass
import concourse.mybir as mybir
import concourse.tile as tile
from concourse import bacc

# ---- constants (hardcoded per the contract; kernel.py must be self-contained)
B, S, D, H = 4, 1024, 1024, 16
DG = 512          # feature slice per core (8 heads)
HL = 8            # heads per core
DH = 64
P = 128
NCORES = 8
NJT = DG // P     # 4 feature tiles == head pairs
NST = S // P      # 8 sequence tiles
NDT = D // P      # 8 contraction tiles for projections
NPC = S // 512    # 2 query chunks of 512

F32 = mybir.dt.float32
BF16 = mybir.dt.bfloat16
FP8 = mybir.dt.float8e4
AF = mybir.ActivationFunctionType
ALU = mybir.AluOpType
DR = mybir.MatmulPerfMode.DoubleRow
NPBF16 = ml_dtypes.bfloat16
NPFP8 = ml_dtypes.float8_e4m3


def build_bass():
    nc = bacc.Bacc("TRN2", target_bir_lowering=False, debug=False,
                   num_devices=NCORES)

    xqT = nc.dram_tensor("xqT", [D, S], FP8, kind="ExternalInput").ap()
    xkT = nc.dram_tensor("xkT", [D, S], FP8, kind="ExternalInput").ap()
    xvT = nc.dram_tensor("xvT", [D, S], FP8, kind="ExternalInput").ap()
    wq = nc.dram_tensor("wq", [D, DG], FP8, kind="ExternalInput").ap()
    wk = nc.dram_tensor("wk", [D, DG], FP8, kind="ExternalInput").ap()
    wv = nc.dram_tensor("wv", [D, DG], FP8, kind="ExternalInput").ap()
    bq = nc.dram_tensor("bq", [1, DG], F32, kind="ExternalInput").ap()
    bk = nc.dram_tensor("bk", [1, DG], F32, kind="ExternalInput").ap()
    bv = nc.dram_tensor("bv", [1, DG], BF16, kind="ExternalInput").ap()
    wo = nc.dram_tensor("wo", [DG, D], BF16, kind="ExternalInput").ap()
    out = nc.dram_tensor("out", [S, D], BF16, kind="ExternalOutput").ap()

    with tile.TileContext(nc) as tc, ExitStack() as ctx, \
            nc.allow_low_precision(reason="bf16 compute is intentional"):
        consts = ctx.enter_context(tc.tile_pool(name="consts", bufs=1))
        xpool = ctx.enter_context(tc.tile_pool(name="xpool", bufs=1))
        qkpool = ctx.enter_context(tc.tile_pool(name="qkpool", bufs=1))
        vpool = ctx.enter_context(tc.tile_pool(name="vpool", bufs=1))
        epool = ctx.enter_context(tc.tile_pool(name="epool", bufs=3))
        aopool = ctx.enter_context(tc.tile_pool(name="aopool", bufs=1))
        t1pool = ctx.enter_context(tc.tile_pool(name="t1pool", bufs=2))
        espool = ctx.enter_context(tc.tile_pool(name="espool", bufs=2))
        rpool = ctx.enter_context(tc.tile_pool(name="rpool", bufs=2))
        zsbpool = ctx.enter_context(tc.tile_pool(name="zsbpool", bufs=2))
        outpool = ctx.enter_context(tc.tile_pool(name="outpool", bufs=2))

        # PSUM: psA 2x[128,1024] (scores + QK proj chains + pc1 out-proj)
        # = 4 banks, psB 2x[128,512] (attn@v accum + V proj chains) = 2,
        # psZD 2x[128,512] (Z staging/broadcast + pc0 out-proj) = 2.
        psA = ctx.enter_context(tc.tile_pool(name="psA", bufs=2, space="PSUM"))
        psB = ctx.enter_context(tc.tile_pool(name="psB", bufs=2, space="PSUM"))
        psZD = ctx.enter_context(tc.tile_pool(name="psZD", bufs=2,
                                              space="PSUM"))

        # --- constants (memset where possible; tiny DMAs otherwise)
        onescol = consts.tile([P, 1], BF16, tag="onescol")
        nc.vector.memset(onescol, 1.0)
        onesrow = consts.tile([1, P], BF16, tag="onesrow")
        nc.vector.memset(onesrow, 1.0)
        bcmask = consts.tile([33, P], BF16, tag="bcmask")
        nc.vector.memset(bcmask, 0.0)
        nc.vector.memset(bcmask[0:1, 0:DH], 1.0)
        nc.vector.memset(bcmask[32:33, DH:P], 1.0)

        bv_sb = consts.tile([1, DG], BF16, tag="bv")
        nc.sync.dma_start(out=bv_sb, in_=bv)
        bqT = consts.tile([P, NJT], F32, tag="bqT")
        nc.sync.dma_start(out=bqT, in_=bq[0, :].rearrange("(jt p) -> p jt", p=P))
        bkT = consts.tile([P, NJT], F32, tag="bkT")
        nc.sync.dma_start(out=bkT, in_=bk[0, :].rearrange("(jt p) -> p jt", p=P))

        # dummy exp to pull the ACT table load off the critical path
        dummy = consts.tile([1, 8], BF16, tag="dummy")
        nc.scalar.activation(dummy, bcmask[0:1, 0:8], AF.Exp)

        # --- inputs: big SBUF tiles, large DMAs in criticality order.
        # HWDGE rings fair-share bandwidth across in-flight DMAs, so the
        # critical set (K + Q jt0 inputs -> first scores block) is issued
        # first and everything else after.
        # gates: tiles holding the V/wo pool slots, released only after the
        # first K/Q wave completes -> those DMAs can't start (and steal
        # fair-shared HBM bandwidth) until the critical inputs have landed.
        gate_xv = xpool.tile([1, 8], BF16, tag="xv", name="gate_xv")
        gate_wv = xpool.tile([1, 8], BF16, tag="wv", name="gate_wv")
        gate_wo = consts.tile([1, 8], BF16, tag="wo3", name="gate_wo")
        gsink = consts.tile([1, 8], BF16, tag="gsink")
        for g in (gate_xv, gate_wv, gate_wo):
            nc.vector.memset(g, 0.0)

        xk_a = xpool.tile([P, NDT, S], FP8, tag="xk")
        xq_a = xpool.tile([P, NDT, S], FP8, tag="xq")
        xv_a = xpool.tile([P, NDT, S], FP8, tag="xv")
        wk_a = xpool.tile([P, NDT, DG], FP8, tag="wk")
        wq_a = xpool.tile([P, NDT, DG], FP8, tag="wq")
        wv_a = xpool.tile([P, NDT, DG], FP8, tag="wv")
        wo3 = consts.tile([P, NJT, D], BF16, tag="wo3")

        def dma_w_jt(dst, w, jt):
            nc.sync.dma_start(
                out=dst[:, :, jt * P:(jt + 1) * P],
                in_=w[:, jt * P:(jt + 1) * P].rearrange(
                    "(f p) g -> p f g", p=P))

        def dma_x(dst, xT):
            for c in range(2):
                nc.sync.dma_start(
                    out=dst[:, 4 * c:4 * c + 4, :],
                    in_=xT[c * 512:(c + 1) * 512, :].rearrange(
                        "(f p) s -> p f s", p=P))

        dma_w_jt(wk_a, wk, 0)
        dma_x(xk_a, xkT)
        dma_w_jt(wq_a, wq, 0)
        dma_x(xq_a, xqT)
        for jt in range(1, NJT):
            dma_w_jt(wk_a, wk, jt)
            dma_w_jt(wq_a, wq, jt)

        def emit_gated_dmas():
            for c in range(2):
                nc.sync.dma_start(
                    out=wv_a[:, 4 * c:4 * c + 4, :],
                    in_=wv[c * 512:(c + 1) * 512, :].rearrange(
                        "(f p) g -> p f g", p=P))
            for c in range(2):
                nc.sync.dma_start(
                    out=xv_a[:, 4 * c:4 * c + 4, :],
                    in_=xvT[c * 512:(c + 1) * 512, :].rearrange(
                        "(f p) s -> p f s", p=P))
            for c in range(2):
                nc.sync.dma_start(
                    out=wo3[:, 2 * c:2 * c + 2, :],
                    in_=wo[c * 256:(c + 1) * 256, :].rearrange(
                        "(f p) d2 -> p f d2", p=P))

        # --- transposed Q/K projections: dst[:, jt, pc*512:] = relu(w.T@x+b)
        # per-jt waves: one K tile (pc0|pc1 halves) + one Q tile, so the
        # hp=jt0 scores block is ready after the first wave.
        qpT = qkpool.tile([P, NJT, S], BF16, tag="qpT")
        kpT = qkpool.tile([P, NJT, S], BF16, tag="kpT")

        def qk_wave(jt):
            # fp8 DoubleRow: each matmul contracts a dt-pair (K=256 virtual)
            tk = psA.tile([P, 1024], F32, tag="ps", name=f"ps_k{jt}")
            tq = psA.tile([P, 1024], F32, tag="ps", name=f"ps_q{jt}")
            for c in range(NDT // 2):
                for pc in range(NPC):
                    nc.tensor.matmul(
                        tk[:, pc * 512:(pc + 1) * 512],
                        lhsT=wk_a[:, 2 * c:2 * c + 2, jt * P:(jt + 1) * P],
                        rhs=xk_a[:, 2 * c:2 * c + 2,
                                 pc * 512:(pc + 1) * 512],
                        start=(c == 0), stop=(c == NDT // 2 - 1),
                        perf_mode=DR)
                    nc.tensor.matmul(
                        tq[:, pc * 512:(pc + 1) * 512],
                        lhsT=wq_a[:, 2 * c:2 * c + 2, jt * P:(jt + 1) * P],
                        rhs=xq_a[:, 2 * c:2 * c + 2,
                                 pc * 512:(pc + 1) * 512],
                        start=(c == 0), stop=(c == NDT // 2 - 1),
                        perf_mode=DR)
            for pc in range(NPC):
                nc.vector.tensor_scalar(
                    out=kpT[:, jt, pc * 512:(pc + 1) * 512],
                    in0=tk[:, pc * 512:(pc + 1) * 512],
                    scalar1=bkT[:, jt:jt + 1], scalar2=0.0,
                    op0=ALU.add, op1=ALU.max)
                nc.vector.tensor_scalar(
                    out=qpT[:, jt, pc * 512:(pc + 1) * 512],
                    in0=tq[:, pc * 512:(pc + 1) * 512],
                    scalar1=bqT[:, jt:jt + 1], scalar2=0.0,
                    op0=ALU.add, op1=ALU.max)

        qk_wave(0)
        # release the DMA gates: these reads depend on the first wave's
        # output, so the V/wo DMAs issue only once the critical set landed.
        nc.vector.tensor_add(gsink, gate_xv, qpT[0:1, 0, 0:8])
        nc.vector.tensor_add(gsink, gate_wv, qpT[0:1, 0, 0:8])
        nc.vector.tensor_add(gsink, gate_wo, qpT[0:1, 0, 0:8])
        emit_gated_dmas()
        # waves jt 1..3 are emitted interleaved with block-0 scores below,
        # so the exp stream starts right after wave 0 instead of queueing
        # behind all projection matmuls in the PE FIFO.

        # --- V projection, natural layout -> vpa [128, st, 512] bf16
        # chains on psB tiles (1 bank each); emitted after the first two
        # scores blocks so it fills PE gaps without starving the exp stream.
        vpa = vpool.tile([P, NST, DG], BF16, tag="vpa")

        def emit_vproj():
            for st in range(NST):
                ps = psB.tile([P, 512], F32, tag="nt", name=f"ps_v{st}")
                for c in range(NDT // 2):
                    nc.tensor.matmul(
                        ps,
                        lhsT=xv_a[:, 2 * c:2 * c + 2, st * P:(st + 1) * P],
                        rhs=wv_a[:, 2 * c:2 * c + 2, :],
                        start=(c == 0), stop=False,
                        perf_mode=DR)
                nc.tensor.matmul(ps, lhsT=onesrow, rhs=bv_sb,
                                 start=False, stop=True)
                nc.vector.tensor_scalar(out=vpa[:, st, :], in0=ps,
                                        scalar1=0.0, scalar2=None, op0=ALU.max)

        # --- attention, software-pipelined across 8 (pc, hp) blocks.
        aoT3 = aopool.tile([P, NJT, S], BF16, tag="aoT3")
        blocks = [(pc, hp) for pc in range(NPC) for hp in range(NJT)]
        ex_tiles = {}

        def setup_block(i):
            ex_tiles[i] = epool.tile([P, NST, 1024], BF16, tag="exp",
                                     name=f"exp{i}")

        def emit_scores_ut(i, ut):
            pc, hp = blocks[i]
            pslice = slice(pc * 512, (pc + 1) * 512)
            ex = ex_tiles[i]
            uslice = slice(ut * P, (ut + 1) * P)
            pw = psA.tile([P, 1024], F32, tag="ps")
            nc.tensor.matmul(
                pw[:, 0:512],
                lhsT=kpT[0:DH, hp, uslice],
                rhs=qpT[0:DH, hp, pslice],
                start=True, stop=True)
            nc.tensor.matmul(
                pw[:, 512:1024],
                lhsT=kpT[DH:P, hp, uslice],
                rhs=qpT[DH:P, hp, pslice],
                start=True, stop=True)
            nc.scalar.activation(ex[:, ut, :], pw, AF.Exp, scale=0.125)

        def emit_scores(i):
            setup_block(i)
            for ut in range(NST):
                emit_scores_ut(i, ut)

        def emit_finish(i):
            pc, hp = blocks[i]
            pslice = slice(pc * 512, (pc + 1) * 512)
            hA, hB = 2 * hp, 2 * hp + 1
            ex = ex_tiles.pop(i)
            # attn @ v: column-paired accumulation over key tiles
            nt = psB.tile([P, 512], F32, tag="nt")
            for ut in range(NST):
                nc.tensor.matmul(
                    nt[0:DH, :],
                    lhsT=vpa[:, ut, hA * DH:(hA + 1) * DH],
                    rhs=ex[:, ut, 0:512],
                    start=(ut == 0), stop=(ut == NST - 1),
                    skip_group_check=True)
                nc.tensor.matmul(
                    nt[DH:P, :],
                    lhsT=vpa[:, ut, hB * DH:(hB + 1) * DH],
                    rhs=ex[:, ut, 512:1024],
                    start=(ut == 0), stop=(ut == NST - 1),
                    skip_group_check=True)
            # softmax denominator: DVE tree-sum over ut, two K=128 matmuls
            # with a ones column reduce partitions -> Z_A (row 0), Z_B (row
            # 32); one [33,512] copy stages both (garbage rows masked by the
            # broadcast matmul).
            t1 = t1pool.tile([P, 4, 1024], BF16, tag="t1")
            # level-1 split in halves: the first add only needs key tiles
            # {0,1,4,5}, so it can run while the last exps still stream
            nc.vector.tensor_add(t1[:, 0:2, :], ex[:, 0:2, :], ex[:, 4:6, :])
            nc.vector.tensor_add(t1[:, 2:4, :], ex[:, 2:4, :], ex[:, 6:8, :])
            nc.vector.tensor_add(t1[:, 0:2, :], t1[:, 0:2, :], t1[:, 2:4, :])
            exsum = espool.tile([P, 1024], BF16, tag="exsum")
            nc.vector.tensor_add(exsum, t1[:, 0, :], t1[:, 1, :])
            zps = psZD.tile([P, 512], F32, tag="po")
            nc.tensor.matmul(zps[0:1, :], lhsT=onescol,
                             rhs=exsum[:, 0:512], start=True, stop=True)
            nc.tensor.matmul(zps[32:33, :], lhsT=onescol,
                             rhs=exsum[:, 512:1024], start=True, stop=True)
            zsb = zsbpool.tile([33, 512], BF16, tag="zsb")
            nc.vector.tensor_copy(zsb, zps[0:33, :])
            # broadcast: rows 0:64 <- Z_A, rows 64:128 <- Z_B
            zbc = psZD.tile([P, 512], F32, tag="po")
            nc.tensor.matmul(zbc, lhsT=bcmask, rhs=zsb,
                             start=True, stop=True)
            rcp = rpool.tile([P, 512], F32, tag="rcp")
            nc.vector.reciprocal_approx_fast(rcp, zbc)
            nc.vector.tensor_mul(aoT3[:, hp, pslice], nt, rcp)

        def emit_outproj(pc):
            # pc0 runs during attention (Z pool, serialized, fills PE gaps);
            # pc1 is the tail: use the freed psA [128,1024] tiles, 2 chains
            # per tile, one wide copy + DMA per pt.
            for pt in range(pc * 4, pc * 4 + 4):
                os_ = outpool.tile([P, 1024], BF16, tag="os")
                if pc == 0:
                    for jj in range(2):
                        po_ = psZD.tile([P, 512], F32, tag="po")
                        for hp in range(NJT):
                            nc.tensor.matmul(
                                po_,
                                lhsT=aoT3[:, hp, pt * P:(pt + 1) * P],
                                rhs=wo3[:, hp, jj * 512:(jj + 1) * 512],
                                start=(hp == 0), stop=(hp == NJT - 1))
                        nc.vector.tensor_copy(
                            os_[:, jj * 512:(jj + 1) * 512], po_)
                nc.sync.dma_start(out=out[pt * P:(pt + 1) * P, :], in_=os_)

        def emit_outproj1_prologue():
            # pc1 out-projection, hp 0..2 partial accumulation: emitted
            # before the last block's finish so these matmuls fill the PE
            # while the final Z/normalize chain runs on DVE.
            po_tiles = []
            for pt in range(4, 8):
                po_ = psA.tile([P, 1024], F32, tag="ps", name=f"ps_o{pt}")
                po_tiles.append(po_)
                for jj in range(2):
                    for hp in range(NJT - 1):
                        nc.tensor.matmul(
                            po_[:, jj * 512:(jj + 1) * 512],
                            lhsT=aoT3[:, hp, pt * P:(pt + 1) * P],
                            rhs=wo3[:, hp, jj * 512:(jj + 1) * 512],
                            start=(hp == 0), stop=False)
            return po_tiles

        def emit_outproj1_finish(po_tiles):
            hp = NJT - 1
            for pt, po_ in zip(range(4, 8), po_tiles):
                os_ = outpool.tile([P, 1024], BF16, tag="os")
                for jj in range(2):
                    nc.tensor.matmul(
                        po_[:, jj * 512:(jj + 1) * 512],
                        lhsT=aoT3[:, hp, pt * P:(pt + 1) * P],
                        rhs=wo3[:, hp, jj * 512:(jj + 1) * 512],
                        start=False, stop=True)
                nc.vector.tensor_copy(os_, po_)
                nc.sync.dma_start(out=out[pt * P:(pt + 1) * P, :], in_=os_)

        # block-0 scores interleave with the remaining projection waves
        # (short FIFO stalls only: each wave gives the exp stream time to
        # drain the psA tiles the next score pair needs).
        setup_block(0)
        emit_scores_ut(0, 0)
        emit_scores_ut(0, 1)
        for jt in range(1, NJT):
            qk_wave(jt)
            emit_scores_ut(0, 2 * jt)
            emit_scores_ut(0, 2 * jt + 1)

        # scores run two blocks ahead of attn@v; the V projection is emitted
        # after the first two scores blocks so the exp stream outranks it.
        nb = len(blocks)
        for i in range(1, nb):
            emit_scores(i)
            if i == 1:
                emit_vproj()
            if i >= 2:
                emit_finish(i - 2)
            if i - 2 == 3:
                emit_outproj(0)
        emit_finish(nb - 2)
        po_tiles = emit_outproj1_prologue()
        emit_finish(nb - 1)
        emit_outproj1_finish(po_tiles)

    nc.compile()
    return nc


_CACHE = {}


def get_nc():
    if "nc" not in _CACHE:
        _CACHE["nc"] = build_bass()
    return _CACHE["nc"]


def make_in_maps(q, k, v, Wq, bq, Wk, bk, Wv, bv, Wo, bo):
    q = np.asarray(q, np.float32)
    k = np.asarray(k, np.float32)
    v = np.asarray(v, np.float32)
    Wq = np.asarray(Wq, np.float32)
    Wk = np.asarray(Wk, np.float32)
    Wv = np.asarray(Wv, np.float32)
    Wo = np.asarray(Wo, np.float32)
    bq = np.asarray(bq, np.float32)
    bk = np.asarray(bk, np.float32)
    bv = np.asarray(bv, np.float32)

    qT = [np.ascontiguousarray(q[b].T).astype(NPFP8) for b in range(B)]
    kT = [np.ascontiguousarray(k[b].T).astype(NPFP8) for b in range(B)]
    vT = [np.ascontiguousarray(v[b].T).astype(NPFP8) for b in range(B)]

    in_maps = []
    for c in range(NCORES):
        b, g = divmod(c, 2)
        sl = slice(g * DG, (g + 1) * DG)
        in_maps.append({
            "xqT": qT[b],
            "xkT": kT[b],
            "xvT": vT[b],
            "wq": np.ascontiguousarray(Wq[:, sl]).astype(NPFP8),
            "wk": np.ascontiguousarray(Wk[:, sl]).astype(NPFP8),
            "wv": np.ascontiguousarray(Wv[:, sl]).astype(NPFP8),
            "bq": np.ascontiguousarray(bq[sl]).reshape(1, DG),
            "bk": np.ascontiguousarray(bk[sl]).reshape(1, DG),
            "bv": np.ascontiguousarray(bv[sl]).reshape(1, DG).astype(NPBF16),
            "wo": np.ascontiguousarray(Wo[sl, :]).astype(NPBF16),
        })
    return in_maps


def combine_outputs(parts, bo):
    bo = np.asarray(bo, np.float32)
    out = np.empty((B, S, D), np.float32)
    for b in range(B):
        p0 = np.asarray(parts[2 * b], np.float32)
        p1 = np.asarray(parts[2 * b + 1], np.float32)
        out[b] = np.maximum(p0 + p1 + bo[None, :], 0.0)
    return out


def run(in_maps, trace=False, **kwargs):
    from concourse.bass_utils import run_bass_kernel_spmd
    nc = get_nc()
    return run_bass_kernel_spmd(nc, in_maps, list(range(NCORES)),
                                trace=trace, **kwargs)


def kernel(q, k, v, Wq, bq, Wk, bk, Wv, bv, Wo, bo):
    in_maps = make_in_maps(q, k, v, Wq, bq, Wk, bk, Wv, bv, Wo, bo)
    res = run(in_maps)
    parts = [res.results[c]["out"] for c in range(NCORES)]
    return combine_outputs(parts, bo)


# revision 39
# speedup vs baseline: 1.0250x; 1.0250x over previous
"""Multi-head attention (Keras-style, relu-activated dense projections)
for Trainium2, SPMD across 8 NeuronCores.

Problem (full shapes):
    B, S, D, H = 4, 1024, 1024, 16 ; DH = 64
    qp = relu(q @ Wq + bq); kp = relu(k @ Wk + bk); vp = relu(v @ Wv + bv)
    per head h: scores = qh @ kh^T / 8 ; attn = softmax(scores)
    out = relu(concat_h(attn @ vh) @ Wo + bo)

Sharding: core c = (batch b = c//2, head-group g = c%2). Each core computes
the 8 heads of group g for batch b end-to-end and produces the partial
output projection  attn_out_g @ Wo[g*512:(g+1)*512, :]  (no bias / relu).
Host sums the two partials per batch, adds bo, applies relu.

v3 design notes:
  - all matmul operands bf16 (host-cast): halves input DMA, FWL weight
    loads, no fp32_mode=HIGH matmuls.
  - inputs land via ~30 large chunked DMAs (512KB) instead of ~95 small
    ones: each dma_start costs ~600ns of serialized HWDGE issue, which
    paced the whole projection era in v2.
  - attention software-pipelined across 8 (pc,hp) blocks: scores+exp of
    block i emitted before attn@v/Z/normalize of block i-1, V projection
    and out-projection emitted as PE filler; Tile's readiness scheduler
    keeps ACT saturated with exp and PE dense (HAM stays warm).
  - engine balance: exp + QK bias-relu + Z staging copy on ACT; tree-sum,
    V relu, reciprocal, normalize, out copies on DVE.
  - partial outputs returned bf16; host combines in fp32.
"""

import numpy as np
import ml_dtypes
from contextlib import ExitStack

import concourse.bass as bass
import concourse.mybir as mybir
import concourse.tile as tile
from concourse import bacc

# ---- constants (hardcoded per the contract; kernel.py must be self-contained)
B, S, D, H = 4, 1024, 1024, 16
DG = 512          # feature slice per core (8 heads)
HL = 8            # heads per core
DH = 64
P = 128
NCORES = 8
NJT = DG // P     # 4 feature tiles == head pairs
NST = S // P      # 8 sequence tiles
NDT = D // P      # 8 contraction tiles for projections
NPC = S // 512    # 2 query chunks of 512

F32 = mybir.dt.float32
BF16 = mybir.dt.bfloat16
FP8 = mybir.dt.float8e4
AF = mybir.ActivationFunctionType
ALU = mybir.AluOpType
DR = mybir.MatmulPerfMode.DoubleRow
NPBF16 = ml_dtypes.bfloat16
NPFP8 = ml_dtypes.float8_e4m3


def build_bass():
    nc = bacc.Bacc("TRN2", target_bir_lowering=False, debug=False,
                   num_devices=NCORES)

    xqT = nc.dram_tensor("xqT", [D, S], FP8, kind="ExternalInput").ap()
    xkT = nc.dram_tensor("xkT", [D, S], FP8, kind="ExternalInput").ap()
    xvT = nc.dram_tensor("xvT", [D, S], FP8, kind="ExternalInput").ap()
    wq = nc.dram_tensor("wq", [D, DG], FP8, kind="ExternalInput").ap()
    wk = nc.dram_tensor("wk", [D, DG], FP8, kind="ExternalInput").ap()
    wv = nc.dram_tensor("wv", [D, DG], FP8, kind="ExternalInput").ap()
    bq = nc.dram_tensor("bq", [1, DG], F32, kind="ExternalInput").ap()
    bk = nc.dram_tensor("bk", [1, DG], F32, kind="ExternalInput").ap()
    bv = nc.dram_tensor("bv", [1, DG], BF16, kind="ExternalInput").ap()
    wo = nc.dram_tensor("wo", [DG, D], BF16, kind="ExternalInput").ap()
    out = nc.dram_tensor("out", [S, D], BF16, kind="ExternalOutput").ap()

    with tile.TileContext(nc) as tc, ExitStack() as ctx, \
            nc.allow_low_precision(reason="bf16 compute is intentional"):
        consts = ctx.enter_context(tc.tile_pool(name="consts", bufs=1))
        xpool = ctx.enter_context(tc.tile_pool(name="xpool", bufs=1))
        qkpool = ctx.enter_context(tc.tile_pool(name="qkpool", bufs=1))
        vpool = ctx.enter_context(tc.tile_pool(name="vpool", bufs=1))
        epool = ctx.enter_context(tc.tile_pool(name="epool", bufs=3))
        aopool = ctx.enter_context(tc.tile_pool(name="aopool", bufs=1))
        t1pool = ctx.enter_context(tc.tile_pool(name="t1pool", bufs=2))
        espool = ctx.enter_context(tc.tile_pool(name="espool", bufs=2))
        rpool = ctx.enter_context(tc.tile_pool(name="rpool", bufs=2))
        zsbpool = ctx.enter_context(tc.tile_pool(name="zsbpool", bufs=2))
        outpool = ctx.enter_context(tc.tile_pool(name="outpool", bufs=2))

        # PSUM: psA 2x[128,1024] (scores + QK proj chains + pc1 out-proj)
        # = 4 banks, psB 2x[128,512] (attn@v accum + V proj chains) = 2,
        # psZD 2x[128,512] (Z staging/broadcast + pc0 out-proj) = 2.
        psA = ctx.enter_context(tc.tile_pool(name="psA", bufs=2, space="PSUM"))
        psB = ctx.enter_context(tc.tile_pool(name="psB", bufs=2, space="PSUM"))
        psZD = ctx.enter_context(tc.tile_pool(name="psZD", bufs=2,
                                              space="PSUM"))

        # --- constants (memset where possible; tiny DMAs otherwise)
        onescol = consts.tile([P, 1], BF16, tag="onescol")
        nc.vector.memset(onescol, 1.0)
        onesrow = consts.tile([1, P], BF16, tag="onesrow")
        nc.vector.memset(onesrow, 1.0)
        bcmask = consts.tile([33, P], BF16, tag="bcmask")
        nc.vector.memset(bcmask, 0.0)
        nc.vector.memset(bcmask[0:1, 0:DH], 1.0)
        nc.vector.memset(bcmask[32:33, DH:P], 1.0)

        bv_sb = consts.tile([1, DG], BF16, tag="bv")
        nc.sync.dma_start(out=bv_sb, in_=bv)
        bqT = consts.tile([P, NJT], F32, tag="bqT")
        nc.sync.dma_start(out=bqT, in_=bq[0, :].rearrange("(jt p) -> p jt", p=P))
        bkT = consts.tile([P, NJT], F32, tag="bkT")
        nc.sync.dma_start(out=bkT, in_=bk[0, :].rearrange("(jt p) -> p jt", p=P))

        # dummy exp to pull the ACT table load off the critical path
        dummy = consts.tile([1, 8], BF16, tag="dummy")
        nc.scalar.activation(dummy, bcmask[0:1, 0:8], AF.Exp)

        # --- inputs: big SBUF tiles, large DMAs in criticality order.
        # HWDGE rings fair-share bandwidth across in-flight DMAs, so the
        # critical set (K + Q jt0 inputs -> first scores block) is issued
        # first and everything else after.
        # gates: tiles holding the V/wo pool slots, released only after the
        # first K/Q wave completes -> those DMAs can't start (and steal
        # fair-shared HBM bandwidth) until the critical inputs have landed.
        gate_xv = xpool.tile([1, 8], BF16, tag="xv", name="gate_xv")
        gate_wv = xpool.tile([1, 8], BF16, tag="wv", name="gate_wv")
        gate_wo = consts.tile([1, 8], BF16, tag="wo3", name="gate_wo")
        gsink = consts.tile([1, 8], BF16, tag="gsink")
        for g in (gate_xv, gate_wv, gate_wo):
            nc.vector.memset(g, 0.0)

        xk_a = xpool.tile([P, NDT, S], FP8, tag="xk")
        xq_a = xpool.tile([P, NDT, S], FP8, tag="xq")
        xv_a = xpool.tile([P, NDT, S], FP8, tag="xv")
        wk_a = xpool.tile([P, NDT, DG], FP8, tag="wk")
        wq_a = xpool.tile([P, NDT, DG], FP8, tag="wq")
        wv_a = xpool.tile([P, NDT, DG], FP8, tag="wv")
        wo3 = consts.tile([P, NJT, D], BF16, tag="wo3")

        def dma_w_jt(dst, w, jt):
            nc.sync.dma_start(
                out=dst[:, :, jt * P:(jt + 1) * P],
                in_=w[:, jt * P:(jt + 1) * P].rearrange(
                    "(f p) g -> p f g", p=P))

        def dma_x(dst, xT):
            for c in range(2):
                nc.sync.dma_start(
                    out=dst[:, 4 * c:4 * c + 4, :],
                    in_=xT[c * 512:(c + 1) * 512, :].rearrange(
                        "(f p) s -> p f s", p=P))

        dma_w_jt(wk_a, wk, 0)
        dma_x(xk_a, xkT)
        dma_w_jt(wq_a, wq, 0)
        dma_x(xq_a, xqT)
        for jt in range(1, NJT):
            dma_w_jt(wk_a, wk, jt)
            dma_w_jt(wq_a, wq, jt)

        def emit_gated_dmas():
            for c in range(2):
                nc.sync.dma_start(
                    out=wv_a[:, 4 * c:4 * c + 4, :],
                    in_=wv[c * 512:(c + 1) * 512, :].rearrange(
                        "(f p) g -> p f g", p=P))
            for c in range(2):
                nc.sync.dma_start(
                    out=xv_a[:, 4 * c:4 * c + 4, :],
                    in_=xvT[c * 512:(c + 1) * 512, :].rearrange(
                        "(f p) s -> p f s", p=P))
            for c in range(2):
                nc.sync.dma_start(
                    out=wo3[:, 2 * c:2 * c + 2, :],
                    in_=wo[c * 256:(c + 1) * 256, :].rearrange(
                        "(f p) d2 -> p f d2", p=P))

        # --- transposed Q/K projections: dst[:, jt, pc*512:] = relu(w.T@x+b)
        # per-jt waves: one K tile (pc0|pc1 halves) + one Q tile, so the
        # hp=jt0 scores block is ready after the first wave.
        qpT = qkpool.tile([P, NJT, S], BF16, tag="qpT")
        kpT = qkpool.tile([P, NJT, S], BF16, tag="kpT")

        def qk_wave(jt):
            # fp8 DoubleRow: each matmul contracts a dt-pair (K=256 virtual)
            tk = psA.tile([P, 1024], F32, tag="ps", name=f"ps_k{jt}")
            tq = psA.tile([P, 1024], F32, tag="ps", name=f"ps_q{jt}")
            for c in range(NDT // 2):
                for pc in range(NPC):
                    nc.tensor.matmul(
                        tk[:, pc * 512:(pc + 1) * 512],
                        lhsT=wk_a[:, 2 * c:2 * c + 2, jt * P:(jt + 1) * P],
                        rhs=xk_a[:, 2 * c:2 * c + 2,
                                 pc * 512:(pc + 1) * 512],
                        start=(c == 0), stop=(c == NDT // 2 - 1),
                        perf_mode=DR)
                    nc.tensor.matmul(
                        tq[:, pc * 512:(pc + 1) * 512],
                        lhsT=wq_a[:, 2 * c:2 * c + 2, jt * P:(jt + 1) * P],
                        rhs=xq_a[:, 2 * c:2 * c + 2,
                                 pc * 512:(pc + 1) * 512],
                        start=(c == 0), stop=(c == NDT // 2 - 1),
                        perf_mode=DR)
            for pc in range(NPC):
                nc.vector.tensor_scalar(
                    out=kpT[:, jt, pc * 512:(pc + 1) * 512],
                    in0=tk[:, pc * 512:(pc + 1) * 512],
                    scalar1=bkT[:, jt:jt + 1], scalar2=0.0,
                    op0=ALU.add, op1=ALU.max)
                nc.vector.tensor_scalar(
                    out=qpT[:, jt, pc * 512:(pc + 1) * 512],
                    in0=tq[:, pc * 512:(pc + 1) * 512],
                    scalar1=bqT[:, jt:jt + 1], scalar2=0.0,
                    op0=ALU.add, op1=ALU.max)

        qk_wave(0)
        # release the (vestigial) gates immediately; V/wo inputs must land
        # early so attn@v(0) never stalls at the PE queue head.
        nc.vector.tensor_add(gsink, gate_xv, gate_wv)
        nc.vector.tensor_add(gsink, gate_wo, gate_wo)
        emit_gated_dmas()
        # waves jt 1..3 are emitted interleaved with block-0 scores below,
        # so the exp stream starts right after wave 0 instead of queueing
        # behind all projection matmuls in the PE FIFO.

        # --- V projection, natural layout -> vpa [128, st, 512] bf16
        # chains on psB tiles (1 bank each); emitted after the first two
        # scores blocks so it fills PE gaps without starving the exp stream.
        vpa = vpool.tile([P, NST, DG], BF16, tag="vpa")

        def emit_vproj():
            for st in range(NST):
                ps = psB.tile([P, 512], F32, tag="nt", name=f"ps_v{st}")
                for c in range(NDT // 2):
                    nc.tensor.matmul(
                        ps,
                        lhsT=xv_a[:, 2 * c:2 * c + 2, st * P:(st + 1) * P],
                        rhs=wv_a[:, 2 * c:2 * c + 2, :],
                        start=(c == 0), stop=False,
                        perf_mode=DR)
                nc.tensor.matmul(ps, lhsT=onesrow, rhs=bv_sb,
                                 start=False, stop=True)
                nc.vector.tensor_scalar(out=vpa[:, st, :], in0=ps,
                                        scalar1=0.0, scalar2=None, op0=ALU.max)

        # --- attention, software-pipelined across 8 (pc, hp) blocks.
        aoT3 = aopool.tile([P, NJT, S], BF16, tag="aoT3")
        blocks = [(pc, hp) for pc in range(NPC) for hp in range(NJT)]
        ex_tiles = {}

        def setup_block(i):
            ex_tiles[i] = epool.tile([P, NST, 1024], BF16, tag="exp",
                                     name=f"exp{i}")

        def emit_scores_ut(i, ut):
            pc, hp = blocks[i]
            pslice = slice(pc * 512, (pc + 1) * 512)
            ex = ex_tiles[i]
            uslice = slice(ut * P, (ut + 1) * P)
            pw = psA.tile([P, 1024], F32, tag="ps")
            nc.tensor.matmul(
                pw[:, 0:512],
                lhsT=kpT[0:DH, hp, uslice],
                rhs=qpT[0:DH, hp, pslice],
                start=True, stop=True)
            nc.tensor.matmul(
                pw[:, 512:1024],
                lhsT=kpT[DH:P, hp, uslice],
                rhs=qpT[DH:P, hp, pslice],
                start=True, stop=True)
            nc.scalar.activation(ex[:, ut, :], pw, AF.Exp, scale=0.125)

        def emit_scores(i):
            setup_block(i)
            for ut in range(NST):
                emit_scores_ut(i, ut)

        def emit_finish(i):
            pc, hp = blocks[i]
            pslice = slice(pc * 512, (pc + 1) * 512)
            hA, hB = 2 * hp, 2 * hp + 1
            ex = ex_tiles.pop(i)
            # attn @ v: column-paired accumulation over key tiles
            nt = psB.tile([P, 512], F32, tag="nt")
            for ut in range(NST):
                nc.tensor.matmul(
                    nt[0:DH, :],
                    lhsT=vpa[:, ut, hA * DH:(hA + 1) * DH],
                    rhs=ex[:, ut, 0:512],
                    start=(ut == 0), stop=(ut == NST - 1),
                    skip_group_check=True)
                nc.tensor.matmul(
                    nt[DH:P, :],
                    lhsT=vpa[:, ut, hB * DH:(hB + 1) * DH],
                    rhs=ex[:, ut, 512:1024],
                    start=(ut == 0), stop=(ut == NST - 1),
                    skip_group_check=True)
            # softmax denominator: DVE tree-sum over ut, two K=128 matmuls
            # with a ones column reduce partitions -> Z_A (row 0), Z_B (row
            # 32); one [33,512] copy stages both (garbage rows masked by the
            # broadcast matmul).
            t1 = t1pool.tile([P, 4, 1024], BF16, tag="t1")
            # level-1 split in halves: the first add only needs key tiles
            # {0,1,4,5}, so it can run while the last exps still stream
            nc.vector.tensor_add(t1[:, 0:2, :], ex[:, 0:2, :], ex[:, 4:6, :])
            nc.vector.tensor_add(t1[:, 2:4, :], ex[:, 2:4, :], ex[:, 6:8, :])
            nc.vector.tensor_add(t1[:, 0:2, :], t1[:, 0:2, :], t1[:, 2:4, :])
            exsum = espool.tile([P, 1024], BF16, tag="exsum")
            nc.vector.tensor_add(exsum, t1[:, 0, :], t1[:, 1, :])
            zps = psZD.tile([P, 512], F32, tag="po")
            nc.tensor.matmul(zps[0:1, :], lhsT=onescol,
                             rhs=exsum[:, 0:512], start=True, stop=True)
            nc.tensor.matmul(zps[32:33, :], lhsT=onescol,
                             rhs=exsum[:, 512:1024], start=True, stop=True)
            zsb = zsbpool.tile([33, 512], BF16, tag="zsb")
            nc.vector.tensor_copy(zsb, zps[0:33, :])
            # broadcast: rows 0:64 <- Z_A, rows 64:128 <- Z_B
            zbc = psZD.tile([P, 512], F32, tag="po")
            nc.tensor.matmul(zbc, lhsT=bcmask, rhs=zsb,
                             start=True, stop=True)
            rcp = rpool.tile([P, 512], F32, tag="rcp")
            nc.vector.reciprocal_approx_fast(rcp, zbc)
            nc.vector.tensor_mul(aoT3[:, hp, pslice], nt, rcp)

        def emit_outproj(pc):
            # pc0 runs during attention (Z pool, serialized, fills PE gaps);
            # pc1 is the tail: use the freed psA [128,1024] tiles, 2 chains
            # per tile, one wide copy + DMA per pt.
            for pt in range(pc * 4, pc * 4 + 4):
                os_ = outpool.tile([P, 1024], BF16, tag="os")
                if pc == 0:
                    for jj in range(2):
                        po_ = psZD.tile([P, 512], F32, tag="po")
                        for hp in range(NJT):
                            nc.tensor.matmul(
                                po_,
                                lhsT=aoT3[:, hp, pt * P:(pt + 1) * P],
                                rhs=wo3[:, hp, jj * 512:(jj + 1) * 512],
                                start=(hp == 0), stop=(hp == NJT - 1))
                        nc.vector.tensor_copy(
                            os_[:, jj * 512:(jj + 1) * 512], po_)
                nc.sync.dma_start(out=out[pt * P:(pt + 1) * P, :], in_=os_)

        def emit_outproj1_prologue():
            # pc1 out-projection, hp 0..2 partial accumulation: emitted
            # before the last block's finish so these matmuls fill the PE
            # while the final Z/normalize chain runs on DVE.
            po_tiles = []
            for pt in range(4, 8):
                po_ = psA.tile([P, 1024], F32, tag="ps", name=f"ps_o{pt}")
                po_tiles.append(po_)
                for jj in range(2):
                    for hp in range(NJT - 1):
                        nc.tensor.matmul(
                            po_[:, jj * 512:(jj + 1) * 512],
                            lhsT=aoT3[:, hp, pt * P:(pt + 1) * P],
                            rhs=wo3[:, hp, jj * 512:(jj + 1) * 512],
                            start=(hp == 0), stop=False)
            return po_tiles

        def emit_outproj1_finish(po_tiles):
            hp = NJT - 1
            for pt, po_ in zip(range(4, 8), po_tiles):
                os_ = outpool.tile([P, 1024], BF16, tag="os")
                for jj in range(2):
                    nc.tensor.matmul(
                        po_[:, jj * 512:(jj + 1) * 512],
                        lhsT=aoT3[:, hp, pt * P:(pt + 1) * P],
                        rhs=wo3[:, hp, jj * 512:(jj + 1) * 512],
                        start=False, stop=True)
                nc.vector.tensor_copy(os_, po_)
                nc.sync.dma_start(out=out[pt * P:(pt + 1) * P, :], in_=os_)

        # block-0 scores interleave with the remaining projection waves
        # (short FIFO stalls only: each wave gives the exp stream time to
        # drain the psA tiles the next score pair needs).
        setup_block(0)
        emit_scores_ut(0, 0)
        emit_scores_ut(0, 1)
        for jt in range(1, NJT):
            qk_wave(jt)
            emit_scores_ut(0, 2 * jt)
            emit_scores_ut(0, 2 * jt + 1)

        # scores run two blocks ahead of attn@v; the V projection is emitted
        # after the first two scores blocks so the exp stream outranks it.
        nb = len(blocks)
        for i in range(1, nb):
            emit_scores(i)
            if i == 1:
                emit_vproj()
            if i >= 2:
                emit_finish(i - 2)
            if i - 2 == 3:
                emit_outproj(0)
        emit_finish(nb - 2)
        po_tiles = emit_outproj1_prologue()
        emit_finish(nb - 1)
        emit_outproj1_finish(po_tiles)

    nc.compile()
    return nc


_CACHE = {}


def get_nc():
    if "nc" not in _CACHE:
        _CACHE["nc"] = build_bass()
    return _CACHE["nc"]


def make_in_maps(q, k, v, Wq, bq, Wk, bk, Wv, bv, Wo, bo):
    q = np.asarray(q, np.float32)
    k = np.asarray(k, np.float32)
    v = np.asarray(v, np.float32)
    Wq = np.asarray(Wq, np.float32)
    Wk = np.asarray(Wk, np.float32)
    Wv = np.asarray(Wv, np.float32)
    Wo = np.asarray(Wo, np.float32)
    bq = np.asarray(bq, np.float32)
    bk = np.asarray(bk, np.float32)
    bv = np.asarray(bv, np.float32)

    qT = [np.ascontiguousarray(q[b].T).astype(NPFP8) for b in range(B)]
    kT = [np.ascontiguousarray(k[b].T).astype(NPFP8) for b in range(B)]
    vT = [np.ascontiguousarray(v[b].T).astype(NPFP8) for b in range(B)]

    in_maps = []
    for c in range(NCORES):
        b, g = divmod(c, 2)
        sl = slice(g * DG, (g + 1) * DG)
        in_maps.append({
            "xqT": qT[b],
            "xkT": kT[b],
            "xvT": vT[b],
            "wq": np.ascontiguousarray(Wq[:, sl]).astype(NPFP8),
            "wk": np.ascontiguousarray(Wk[:, sl]).astype(NPFP8),
            "wv": np.ascontiguousarray(Wv[:, sl]).astype(NPFP8),
            "bq": np.ascontiguousarray(bq[sl]).reshape(1, DG),
            "bk": np.ascontiguousarray(bk[sl]).reshape(1, DG),
            "bv": np.ascontiguousarray(bv[sl]).reshape(1, DG).astype(NPBF16),
            "wo": np.ascontiguousarray(Wo[sl, :]).astype(NPBF16),
        })
    return in_maps


def combine_outputs(parts, bo):
    bo = np.asarray(bo, np.float32)
    out = np.empty((B, S, D), np.float32)
    for b in range(B):
        p0 = np.asarray(parts[2 * b], np.float32)
        p1 = np.asarray(parts[2 * b + 1], np.float32)
        out[b] = np.maximum(p0 + p1 + bo[None, :], 0.0)
    return out


def run(in_maps, trace=False, **kwargs):
    from concourse.bass_utils import run_bass_kernel_spmd
    nc = get_nc()
    return run_bass_kernel_spmd(nc, in_maps, list(range(NCORES)),
                                trace=trace, **kwargs)


def kernel(q, k, v, Wq, bq, Wk, bk, Wv, bv, Wo, bo):
    in_maps = make_in_maps(q, k, v, Wq, bq, Wk, bk, Wv, bv, Wo, bo)
    res = run(in_maps)
    parts = [res.results[c]["out"] for c in range(NCORES)]
    return combine_outputs(parts, bo)
